# revision 35
# baseline (speedup 1.0000x reference)
"""Trainium2 Bass kernel for the ConvolutionalCapsule module.

Sharding: data-parallel over (batch, H-half): core k handles b = k//2,
output rows h in [6*(k%2), 6*(k%2)+6), i.e. 72 spatial positions per core.
Weights replicated; host only does layout (patch extraction + W transposes).

Layout (per core): partition p = c_lo*8 + i with c = 16*t + c_lo,
t in [0,18) chunks; free index col = t*72 + pos ("slab" width 1296).

Device algorithm per core:
  B:    o0p[pos,(f,o)] = sum_{(c,i)} P*W via 18 accumulating matmuls (N=512);
        out0 = squash(o0p/F); PE-transpose -> out0T fp8 [(j,o), (g,pos)].
  V:    per f: 18 matmuls  V[(c,i),pos] = sum_o W*out0  (fp8 lhsT [32,128]
        at row-group j=f%4, f-rotated emission across a quad of 4 f's).
  VP:   DVE tensor_mul (V psum fp32 x P fp16 -> fp16), per 432-col sub.
  agr:  one [128,128] 0/1 matrix ("smat") per sub: sums i and replicates
        the result over i in a single matmul; quad g's reduce work is
        emitted after quad g+1's V-matmuls (software pipelining).
  e:    ACT exp from agr psum -> e_all fp16.
  Z:    two running-sum chains (gpsimd: f 0..15, DVE: f 16..31), merged,
        then Zr = exp(-ln(Z)) on ACT, pp = P*Zr -- per-sub pipelined.
  cen:  S_f = pp * e_f (DVE, one quad ahead); per f: 18 matmuls
        (lhsT w_r [128,16], col-group j=f%4) -> squash -> y.
"""
import numpy as np

KH = KW = 3
B, H, WD, FIN, DIN = 4, 14, 14, 32, 8
F, C, DO, DI = 32, 288, 16, 8
NPOS = 72
NT = 18            # c-chunks of 16
CB = 16            # c per chunk
SLAB = NT * NPOS   # 1296
NSUB = 3
SUBT = NT // NSUB  # 6 chunks per sub
SUBW = SUBT * NPOS # 432
EPS = 1e-7
NGP_S = 0          # S-muls done on gpsimd (rest on DVE)
ACT_V_MOD = 0      # every ACT_V_MOD-th V-sub drains via ACT (0 = never)

_CACHE: dict = {}


def _host_weights(Wm):
    """Wm: [F, C, DO, DI] float32 -> (w_r, w_v) fp16 device layouts."""
    # w_r[c_lo*8+i, t*512 + f*16 + o] = Wm[f, 16t+c_lo, o, i]
    w_r = (
        Wm.transpose(1, 3, 0, 2)          # [C, DI, F, DO]
        .reshape(NT, CB, DI, F * DO)
        .reshape(NT, 128, F * DO)
        .transpose(1, 0, 2)
        .reshape(128, NT * F * DO)
        .astype(np.float16)
        .copy()
    )
    # w_v[32*(f%4)+o, ((f//4)*NT+t)*128 + c_lo*8+i] = Wm[f, 16t+c_lo, o, i]
    import ml_dtypes
    w_v = np.zeros((128, 8 * NT * 128), np.float32)
    for f in range(F):
        g, j = divmod(f, 4)
        arr = (
            Wm[f].reshape(NT, CB, DO, DI)  # [t, c_lo, o, i]
            .transpose(2, 0, 1, 3)
            .reshape(DO, NT, 128)
        )
        w_v[32 * j:32 * j + DO, (g * NT) * 128:(g + 1) * NT * 128] = (
            arr.reshape(DO, NT * 128)
        )
    return w_r, w_v.astype(ml_dtypes.float8_e4m3)


def _host_patches(x, k):
    """Patch tensor for core k: [128, SLAB] fp16, p=(c_lo,i), col=(t,pos)."""
    b, hh = divmod(k, 2)
    h0 = 6 * hh
    P = np.empty((6, 12, KH, KW, FIN, DIN), np.float32)
    for kh in range(KH):
        for kw in range(KW):
            for h in range(6):
                P[h, :, kh, kw] = x[b, h0 + h + kh, kw:kw + 12]
    P = P.reshape(NPOS, C, DIN)            # [pos, c, i]
    p_ct = (
        P.reshape(NPOS, NT, CB, DIN)
        .transpose(1, 2, 3, 0)             # [t, c_lo, i, pos]
        .reshape(NT, 128, NPOS)
        .transpose(1, 0, 2)
        .reshape(128, SLAB)
        .astype(np.float16)
        .copy()
    )
    return p_ct


def _build():
    import concourse.bass as bass
    import concourse.bacc as bacc
    import concourse.mybir as mybir
    import concourse.tile as tile
    from concourse.alu_op_type import AluOpType

    F16, F32 = mybir.dt.float16, mybir.dt.float32
    F8 = mybir.dt.float8e4
    AX = mybir.AxisListType
    AF = mybir.ActivationFunctionType

    nc = bacc.Bacc(None, target_bir_lowering=False, debug=False)

    p_ct_d = nc.dram_tensor("p_ct", [128, SLAB], F16, kind="ExternalInput")
    w_r_d = nc.dram_tensor("w_r", [128, NT * F * DO], F16, kind="ExternalInput")
    w_v_d = nc.dram_tensor("w_v", [128, 8 * NT * 128], F8, kind="ExternalInput")
    smat_d = nc.dram_tensor("smat", [128, 128], F16, kind="ExternalInput")
    eye72_d = nc.dram_tensor("eye72", [NPOS, NPOS], F16, kind="ExternalInput")
    eye128f_d = nc.dram_tensor("eye128f", [128, 128], F32, kind="ExternalInput")
    y_d = nc.dram_tensor("y", [NPOS, F * DO], F32, kind="ExternalOutput")

    with tile.TileContext(nc) as tc:
        with (
            tc.tile_pool(name="const", bufs=1) as const,
            tc.tile_pool(name="work", bufs=1) as work,
            tc.tile_pool(name="ring", bufs=4) as ring,
            tc.tile_pool(name="vps", bufs=4, space=bass.MemorySpace.PSUM) as vps,
            tc.tile_pool(name="agp", bufs=4, space=bass.MemorySpace.PSUM) as agp,
        ):
            # ---------------- loads ----------------
            p_ct = const.tile([128, SLAB], F16, tag="p_ct")
            nc.sync.dma_start(p_ct[:], p_ct_d[:])
            w_r = const.tile([128, NT * F * DO], F16, tag="w_r")
            for q in range(6):
                sl = slice(q * 3 * F * DO, (q + 1) * 3 * F * DO)
                nc.sync.dma_start(w_r[:, sl], w_r_d[:, sl])
            w_v = const.tile([128, 8 * NT * 128], F8, tag="w_v")
            for g in range(8):
                sl = slice(g * NT * 128, (g + 1) * NT * 128)
                nc.sync.dma_start(w_v[:, sl], w_v_d[:, sl])
            smat = const.tile([128, 128], F16, tag="smat")
            nc.sync.dma_start(smat[:], smat_d[:])
            eye72 = const.tile([NPOS, NPOS], F16, tag="eye72")
            nc.sync.dma_start(eye72[:], eye72_d[:])
            eye128f = const.tile([128, 128], F32, tag="eye128f")
            nc.sync.dma_start(eye128f[:], eye128f_d[:])

            def squash(src_ap, dst_ap, pre_scale, tag):
                """dst = squash(src * pre_scale); src/dst free = (f,o)=512."""
                s = work.tile([NPOS, F * DO], F32, tag=f"{tag}_s")
                nc.scalar.activation(s[:], src_ap, AF.Copy, scale=pre_scale)
                sq = work.tile([NPOS, F * DO], F32, tag=f"{tag}_sq")
                nc.scalar.activation(sq[:], s[:], AF.Square)
                sn = work.tile([NPOS, F], F32, tag=f"{tag}_sn")
                nc.vector.reduce_sum(
                    sn[:], sq[:].rearrange("p (f o) -> p f o", o=DO), axis=AX.X
                )
                t1 = work.tile([NPOS, F], F32, tag=f"{tag}_t1")
                nc.vector.tensor_scalar_add(t1[:], sn[:], 1.0)
                r1 = work.tile([NPOS, F], F32, tag=f"{tag}_r1")
                nc.vector.reciprocal(r1[:], t1[:])
                se = work.tile([NPOS, F], F32, tag=f"{tag}_se")
                nc.vector.tensor_scalar_add(se[:], sn[:], EPS)
                r2 = work.tile([NPOS, F], F32, tag=f"{tag}_r2")
                nc.scalar.activation(r2[:], se[:], AF.Sqrt)
                r3 = work.tile([NPOS, F], F32, tag=f"{tag}_r3")
                nc.vector.reciprocal(r3[:], r2[:])
                sc = work.tile([NPOS, F], F32, tag=f"{tag}_sc")
                nc.vector.tensor_mul(sc[:], sn[:], r1[:])
                sc2 = work.tile([NPOS, F], F32, tag=f"{tag}_sc2")
                nc.vector.tensor_mul(sc2[:], sc[:], r3[:])
                bc = sc2[:].unsqueeze(2).broadcast_to((NPOS, F, DO))
                nc.vector.tensor_mul(
                    dst_ap, s[:].rearrange("p (f o) -> p f o", o=DO), bc
                )

            # ---------------- stage B: out0 ----------------
            o0p = vps.tile([NPOS, F * DO], F32, tag="vps")
            for t in range(NT):
                nc.tensor.matmul(
                    o0p[:],
                    p_ct[:, t * NPOS:(t + 1) * NPOS],
                    w_r[:, t * F * DO:(t + 1) * F * DO],
                    start=(t == 0),
                    stop=(t == NT - 1),
                )
            out0_pad = work.tile([NPOS, F * 32], F16, tag="out0_pad")
            nc.vector.memset(out0_pad[:], 0.0)
            squash(
                o0p[:],
                out0_pad[:].rearrange("p (f s) -> p f s", s=32)[:, :, 0:DO],
                1.0 / F,
                "sq1",
            )
            # transposes -> out0T fp16 [128=(j,o-slot), 8*72]
            out0T = work.tile([128, 8 * NPOS], F8, tag="out0T")
            for g in range(8):
                tp = vps.tile([128, 128], F16, tag="vps")
                nc.tensor.transpose(
                    tp[:, 0:NPOS], out0_pad[:, g * 128:(g + 1) * 128], eye72[:]
                )
                nc.scalar.copy(out0T[:, g * NPOS:(g + 1) * NPOS], tp[:, 0:NPOS])

            # ---------------- pass A: V, VP, agr, e ----------------
            # Z accumulation rides along: after quad g's exps, its 4 e-slabs
            # are summed into zg[g] (SWDGE accumulate-DMA, WAW-ordered), and
            # zg[g] immediately folds into the running Zacc chain.
            e_all = work.tile([128, F * SLAB], F16, tag="e_all")
            # Z runs as two parallel running-sum chains (gpsimd: f 0..15,
            # DVE: f 16..31), merged at the end.
            Zlo = work.tile([128, SLAB], F16, tag="Zlo")
            Zacc = work.tile([128, SLAB], F16, tag="Zacc")
            def emit_v_quad(g):
                """V-matmuls + VP muls for quad g; returns the VPf tiles."""
                VPf = [
                    ring.tile([128, SLAB], F16, tag="VPf", bufs=8,
                              name=f"VPf{g}_{j}")
                    for j in range(4)
                ]
                for s in range(NSUB):
                    vt = [
                        vps.tile([128, SUBW], F32, tag="vps", name=f"vt{g}{s}{j}")
                        for j in range(4)
                    ]
                    for u in range(SUBT):
                        t = s * SUBT + u
                        for j in range(4):
                            nc.tensor.matmul(
                                vt[j][:, u * NPOS:(u + 1) * NPOS],
                                w_v[32 * j:32 * (j + 1),
                                    (g * NT + t) * 128:(g * NT + t + 1) * 128],
                                out0T[32 * j:32 * (j + 1),
                                      g * NPOS:(g + 1) * NPOS],
                                start=True,
                                stop=True,
                                tile_position=(32 * j, 0),
                            )
                    for j in range(4):
                        nc.vector.tensor_mul(
                            VPf[j][:, s * SUBW:(s + 1) * SUBW],
                            vt[j][:],
                            p_ct[:, s * SUBW:(s + 1) * SUBW],
                        )
                return VPf

            def emit_agr_quad(g, VPf):
                """Smat reduce + exp + Z-chain folds for quad g."""
                for j in range(4):
                    f = 4 * g + j
                    for s in range(NSUB):
                        ag = agp.tile([128, SUBW], F32, tag="agr",
                                      name=f"ag{g}{j}{s}")
                        nc.tensor.matmul(
                            ag[:],
                            smat[:],
                            VPf[j][:, s * SUBW:(s + 1) * SUBW],
                            start=True,
                            stop=True,
                        )
                        nc.scalar.activation(
                            e_all[:, f * SLAB + s * SUBW:
                                  f * SLAB + (s + 1) * SUBW],
                            ag[:],
                            AF.Exp,
                        )
                for j in range(4):
                    f = 4 * g + j
                    ef = e_all[:, f * SLAB:(f + 1) * SLAB]
                    if f < 16:
                        if f == 0:
                            nc.gpsimd.tensor_copy(Zlo[:], ef)
                        else:
                            nc.gpsimd.tensor_add(Zlo[:], Zlo[:], ef)
                    else:
                        if f == 16:
                            nc.vector.tensor_copy(Zacc[:], ef)
                        else:
                            nc.vector.tensor_add(Zacc[:], Zacc[:], ef)

            # software-pipelined: quad g's reduce work is emitted after
            # quad g+1's V-matmuls so the in-order Tensor queue never
            # stalls waiting on the DVE VP muls.
            pend = None
            for g in range(8):
                VPf = emit_v_quad(g)
                if pend is not None:
                    emit_agr_quad(g - 1, pend)
                pend = VPf
            emit_agr_quad(7, pend)
            # Zr = 1/Z as exp(-ln(Z)) on ACT (Reciprocal AF is banned and the
            # DVE iterative reciprocal costs ~8us at this size). Merge, Ln,
            # Exp, pp run per 432-col sub so the chain pipelines.
            zln = work.tile([128, SLAB], F32, tag="zln")
            Zr = work.tile([128, SLAB], F16, tag="Zr")
            pp = work.tile([128, SLAB], F16, tag="pp")
            for s in range(NSUB):
                sl = slice(s * SUBW, (s + 1) * SUBW)
                nc.vector.tensor_add(Zacc[:, sl], Zacc[:, sl], Zlo[:, sl])
                nc.scalar.activation(zln[:, sl], Zacc[:, sl], AF.Ln)
                nc.scalar.activation(Zr[:, sl], zln[:, sl], AF.Exp, scale=-1.0)
                nc.vector.tensor_mul(pp[:, sl], p_ct[:, sl], Zr[:, sl])

            # ---------------- pass B: S, centroids ----------------
            cen_sb = work.tile([128, 8 * NPOS], F32, tag="cen_sb")
            gat = work.tile([NPOS, F * DO], F32, tag="gat")

            def emit_s_quad(g):
                S = [
                    ring.tile([128, SLAB], F16, tag="S", bufs=12,
                              name=f"S{g}_{j}")
                    for j in range(4)
                ]
                for j in range(4):
                    f = 4 * g + j
                    nc.vector.tensor_mul(
                        S[j][:], pp[:], e_all[:, f * SLAB:(f + 1) * SLAB]
                    )
                return S

            def emit_cen_quad(g, S):
                cg = vps.tile([128, NPOS], F32, tag="vps", name=f"cg{g}")
                for t in range(NT):
                    for j in range(4):
                        f = 4 * g + j
                        nc.tensor.matmul(
                            cg[32 * j:32 * j + DO, :],
                            w_r[:, t * F * DO + f * DO:t * F * DO + (f + 1) * DO],
                            S[j][:, t * NPOS:(t + 1) * NPOS],
                            start=(t == 0),
                            stop=(t == NT - 1),
                            tile_position=(0, 32 * j),
                        )
                nc.scalar.copy(cen_sb[:, g * NPOS:(g + 1) * NPOS], cg[:])
                # transpose quad g's centroids while quad g+1's matmuls run
                tp2 = agp.tile([128, 128], F32, tag="agr", name=f"tp2_{g}")
                nc.tensor.transpose(
                    tp2[0:NPOS, :], cen_sb[:, g * NPOS:(g + 1) * NPOS],
                    eye128f[:]
                )
                nc.scalar.copy(
                    gat[:, g * 4 * DO:(g + 1) * 4 * DO]
                    .rearrange("p (j o) -> p j o", o=DO),
                    tp2[0:NPOS, :].rearrange("p (j s) -> p j s", s=32)[:, :, 0:DO],
                )

            # one-quad-ahead pipeline: S(q+1) on DVE runs while cen(q)
            # streams on the Tensor engine.
            pend_s = emit_s_quad(0)
            for g in range(8):
                nxt = emit_s_quad(g + 1) if g < 7 else None
                emit_cen_quad(g, pend_s)
                pend_s = nxt

            # ---------------- squash2 + output ----------------
            y_sb = work.tile([NPOS, F * DO], F32, tag="y_sb")
            squash(gat[:], y_sb[:].rearrange("p (f o) -> p f o", o=DO), 1.0, "sq2")
            nc.sync.dma_start(y_d[:], y_sb[:])

    nc.compile()
    return nc


def _get_program():
    if "nc" not in _CACHE:
        _CACHE["nc"] = _build()
    return _CACHE["nc"]


def _host_inputs(x, Wm):
    w_r, w_v = _host_weights(Wm)
    smat = np.kron(np.eye(CB), np.ones((DI, DI))).astype(np.float16)
    eye72 = np.eye(NPOS, dtype=np.float16)
    eye128f = np.eye(128, dtype=np.float32)
    in_maps = []
    for k in range(8):
        in_maps.append({
            "p_ct": _host_patches(x, k),
            "w_r": w_r,
            "w_v": w_v,
            "smat": smat,
            "eye72": eye72,
            "eye128f": eye128f,
        })
    return in_maps


def kernel(x, W):
    from concourse.bass_utils import run_bass_kernel_spmd

    x = np.asarray(x, np.float32)
    Wm = np.asarray(W, np.float32)[0, 0, 0]
    nc = _get_program()
    in_maps = _host_inputs(x, Wm)
    res = run_bass_kernel_spmd(nc, in_maps, list(range(8)))
    Ho, Wo = H - KH + 1, WD - KW + 1
    y = np.empty((B, Ho, Wo, F, DO), np.float32)
    for k in range(8):
        b, hh = divmod(k, 2)
        y[b, 6 * hh:6 * hh + 6] = res.results[k]["y"].reshape(6, Wo, F, DO)
    return y


# revision 36
# speedup vs baseline: 1.0240x; 1.0240x over previous
"""Trainium2 Bass kernel for the ConvolutionalCapsule module.

Sharding: data-parallel over (batch, H-half): core k handles b = k//2,
output rows h in [6*(k%2), 6*(k%2)+6), i.e. 72 spatial positions per core.
Weights replicated; host only does layout (patch extraction + W transposes).

Layout (per core): partition p = c_lo*8 + i with c = 16*t + c_lo,
t in [0,18) chunks; free index col = t*72 + pos ("slab" width 1296).

Device algorithm per core:
  B:    o0p[pos,(f,o)] = sum_{(c,i)} P*W via 18 accumulating matmuls (N=512);
        out0 = squash(o0p/F); PE-transpose -> out0T fp8 [(j,o), (g,pos)].
  V:    per f: 18 matmuls  V[(c,i),pos] = sum_o W*out0  (fp8 lhsT [32,128]
        at row-group j=f%4, f-rotated emission across a quad of 4 f's).
  VP:   DVE tensor_mul (V psum fp32 x P fp16 -> fp16), per 432-col sub.
  agr:  one [128,128] 0/1 matrix ("smat") per sub: sums i and replicates
        the result over i in a single matmul; quad g's reduce work is
        emitted after quad g+1's V-matmuls (software pipelining).
  e:    ACT exp from agr psum -> e_all fp16.
  Z:    two running-sum chains (gpsimd: f 0..15, DVE: f 16..31), merged,
        then Zr = exp(-ln(Z)) on ACT, pp = P*Zr -- per-sub pipelined.
  cen:  S_f = pp * e_f (DVE, one quad ahead); per f: 18 matmuls
        (lhsT w_r [128,16], col-group j=f%4) -> squash -> y.
"""
import numpy as np

KH = KW = 3
B, H, WD, FIN, DIN = 4, 14, 14, 32, 8
F, C, DO, DI = 32, 288, 16, 8
NPOS = 72
NT = 18            # c-chunks of 16
CB = 16            # c per chunk
SLAB = NT * NPOS   # 1296
NSUB = 3
SUBT = NT // NSUB  # 6 chunks per sub
SUBW = SUBT * NPOS # 432
EPS = 1e-7
NGP_S = 0          # S-muls done on gpsimd (rest on DVE)
ACT_V_MOD = 0      # every ACT_V_MOD-th V-sub drains via ACT (0 = never)

_CACHE: dict = {}


def _host_weights(Wm):
    """Wm: [F, C, DO, DI] float32 -> (w_r, w_v) fp16 device layouts."""
    # w_r[c_lo*8+i, t*512 + f*16 + o] = Wm[f, 16t+c_lo, o, i]
    w_r = (
        Wm.transpose(1, 3, 0, 2)          # [C, DI, F, DO]
        .reshape(NT, CB, DI, F * DO)
        .reshape(NT, 128, F * DO)
        .transpose(1, 0, 2)
        .reshape(128, NT * F * DO)
        .astype(np.float16)
        .copy()
    )
    # w_v[32*(f%4)+o, ((f//4)*NT+t)*128 + c_lo*8+i] = Wm[f, 16t+c_lo, o, i]
    import ml_dtypes
    w_v = np.zeros((128, 8 * NT * 128), np.float32)
    for f in range(F):
        g, j = divmod(f, 4)
        arr = (
            Wm[f].reshape(NT, CB, DO, DI)  # [t, c_lo, o, i]
            .transpose(2, 0, 1, 3)
            .reshape(DO, NT, 128)
        )
        w_v[32 * j:32 * j + DO, (g * NT) * 128:(g + 1) * NT * 128] = (
            arr.reshape(DO, NT * 128)
        )
    return w_r, w_v.astype(ml_dtypes.float8_e4m3)


def _host_patches(x, k):
    """Patch tensor for core k: [128, SLAB] fp16, p=(c_lo,i), col=(t,pos)."""
    b, hh = divmod(k, 2)
    h0 = 6 * hh
    P = np.empty((6, 12, KH, KW, FIN, DIN), np.float32)
    for kh in range(KH):
        for kw in range(KW):
            for h in range(6):
                P[h, :, kh, kw] = x[b, h0 + h + kh, kw:kw + 12]
    P = P.reshape(NPOS, C, DIN)            # [pos, c, i]
    p_ct = (
        P.reshape(NPOS, NT, CB, DIN)
        .transpose(1, 2, 3, 0)             # [t, c_lo, i, pos]
        .reshape(NT, 128, NPOS)
        .transpose(1, 0, 2)
        .reshape(128, SLAB)
        .astype(np.float16)
        .copy()
    )
    return p_ct


def _build():
    import concourse.bass as bass
    import concourse.bacc as bacc
    import concourse.mybir as mybir
    import concourse.tile as tile
    from concourse.alu_op_type import AluOpType

    F16, F32 = mybir.dt.float16, mybir.dt.float32
    F8 = mybir.dt.float8e4
    AX = mybir.AxisListType
    AF = mybir.ActivationFunctionType

    nc = bacc.Bacc(None, target_bir_lowering=False, debug=False)

    p_ct_d = nc.dram_tensor("p_ct", [128, SLAB], F16, kind="ExternalInput")
    w_r_d = nc.dram_tensor("w_r", [128, NT * F * DO], F16, kind="ExternalInput")
    w_v_d = nc.dram_tensor("w_v", [128, 8 * NT * 128], F8, kind="ExternalInput")
    smat_d = nc.dram_tensor("smat", [128, 128], F16, kind="ExternalInput")
    eye72_d = nc.dram_tensor("eye72", [NPOS, NPOS], F16, kind="ExternalInput")
    eye128f_d = nc.dram_tensor("eye128f", [128, 128], F32, kind="ExternalInput")
    y_d = nc.dram_tensor("y", [NPOS, F * DO], F32, kind="ExternalOutput")

    with tile.TileContext(nc) as tc:
        with (
            tc.tile_pool(name="const", bufs=1) as const,
            tc.tile_pool(name="work", bufs=1) as work,
            tc.tile_pool(name="ring", bufs=4) as ring,
            tc.tile_pool(name="vps", bufs=4, space=bass.MemorySpace.PSUM) as vps,
            tc.tile_pool(name="agp", bufs=4, space=bass.MemorySpace.PSUM) as agp,
        ):
            # ---------------- loads ----------------
            p_ct = const.tile([128, SLAB], F16, tag="p_ct")
            nc.sync.dma_start(p_ct[:], p_ct_d[:])
            w_r = const.tile([128, NT * F * DO], F16, tag="w_r")
            for q in range(6):
                sl = slice(q * 3 * F * DO, (q + 1) * 3 * F * DO)
                nc.sync.dma_start(w_r[:, sl], w_r_d[:, sl])
            w_v = const.tile([128, 8 * NT * 128], F8, tag="w_v")
            for g in range(8):
                sl = slice(g * NT * 128, (g + 1) * NT * 128)
                nc.sync.dma_start(w_v[:, sl], w_v_d[:, sl])
            smat = const.tile([128, 128], F16, tag="smat")
            nc.sync.dma_start(smat[:], smat_d[:])
            eye72 = const.tile([NPOS, NPOS], F16, tag="eye72")
            nc.sync.dma_start(eye72[:], eye72_d[:])
            eye128f = const.tile([128, 128], F32, tag="eye128f")
            nc.sync.dma_start(eye128f[:], eye128f_d[:])

            def squash(src_ap, dst_ap, pre_scale, tag):
                """dst = squash(src * pre_scale); src/dst free = (f,o)=512."""
                s = work.tile([NPOS, F * DO], F32, tag=f"{tag}_s")
                nc.scalar.activation(s[:], src_ap, AF.Copy, scale=pre_scale)
                sq = work.tile([NPOS, F * DO], F32, tag=f"{tag}_sq")
                nc.scalar.activation(sq[:], s[:], AF.Square)
                sn = work.tile([NPOS, F], F32, tag=f"{tag}_sn")
                nc.vector.reduce_sum(
                    sn[:], sq[:].rearrange("p (f o) -> p f o", o=DO), axis=AX.X
                )
                t1 = work.tile([NPOS, F], F32, tag=f"{tag}_t1")
                nc.vector.tensor_scalar_add(t1[:], sn[:], 1.0)
                r1 = work.tile([NPOS, F], F32, tag=f"{tag}_r1")
                nc.vector.reciprocal(r1[:], t1[:])
                se = work.tile([NPOS, F], F32, tag=f"{tag}_se")
                nc.vector.tensor_scalar_add(se[:], sn[:], EPS)
                r2 = work.tile([NPOS, F], F32, tag=f"{tag}_r2")
                nc.scalar.activation(r2[:], se[:], AF.Sqrt)
                r3 = work.tile([NPOS, F], F32, tag=f"{tag}_r3")
                nc.vector.reciprocal(r3[:], r2[:])
                sc = work.tile([NPOS, F], F32, tag=f"{tag}_sc")
                nc.vector.tensor_mul(sc[:], sn[:], r1[:])
                sc2 = work.tile([NPOS, F], F32, tag=f"{tag}_sc2")
                nc.vector.tensor_mul(sc2[:], sc[:], r3[:])
                bc = sc2[:].unsqueeze(2).broadcast_to((NPOS, F, DO))
                nc.vector.tensor_mul(
                    dst_ap, s[:].rearrange("p (f o) -> p f o", o=DO), bc
                )

            # ---------------- stage B: out0 ----------------
            o0p = vps.tile([NPOS, F * DO], F32, tag="vps")
            for t in range(NT):
                nc.tensor.matmul(
                    o0p[:],
                    p_ct[:, t * NPOS:(t + 1) * NPOS],
                    w_r[:, t * F * DO:(t + 1) * F * DO],
                    start=(t == 0),
                    stop=(t == NT - 1),
                )
            out0_pad = work.tile([NPOS, F * 32], F16, tag="out0_pad")
            nc.vector.memset(out0_pad[:], 0.0)
            squash(
                o0p[:],
                out0_pad[:].rearrange("p (f s) -> p f s", s=32)[:, :, 0:DO],
                1.0 / F,
                "sq1",
            )
            # transposes -> out0T fp16 [128=(j,o-slot), 8*72]
            out0T = work.tile([128, 8 * NPOS], F8, tag="out0T")
            for g in range(8):
                tp = vps.tile([128, 128], F16, tag="vps")
                nc.tensor.transpose(
                    tp[:, 0:NPOS], out0_pad[:, g * 128:(g + 1) * 128], eye72[:]
                )
                nc.scalar.copy(out0T[:, g * NPOS:(g + 1) * NPOS], tp[:, 0:NPOS])

            # ---------------- pass A: V, VP, agr, e ----------------
            # Z accumulation rides along: after quad g's exps, its 4 e-slabs
            # are summed into zg[g] (SWDGE accumulate-DMA, WAW-ordered), and
            # zg[g] immediately folds into the running Zacc chain.
            e_all = work.tile([128, F * SLAB], F16, tag="e_all")
            # Z runs as two parallel running-sum chains (gpsimd: f 0..15,
            # DVE: f 16..31), merged at the end.
            Zlo = work.tile([128, SLAB], F16, tag="Zlo")
            Zacc = work.tile([128, SLAB], F16, tag="Zacc")
            def emit_v_quad(g):
                """V-matmuls + VP muls for quad g; returns the VPf tiles."""
                VPf = [
                    ring.tile([128, SLAB], F16, tag="VPf", bufs=8,
                              name=f"VPf{g}_{j}")
                    for j in range(4)
                ]
                for s in range(NSUB):
                    vt = [
                        vps.tile([128, SUBW], F32, tag="vps", name=f"vt{g}{s}{j}")
                        for j in range(4)
                    ]
                    for u in range(SUBT):
                        t = s * SUBT + u
                        for j in range(4):
                            nc.tensor.matmul(
                                vt[j][:, u * NPOS:(u + 1) * NPOS],
                                w_v[32 * j:32 * (j + 1),
                                    (g * NT + t) * 128:(g * NT + t + 1) * 128],
                                out0T[32 * j:32 * (j + 1),
                                      g * NPOS:(g + 1) * NPOS],
                                start=True,
                                stop=True,
                                tile_position=(32 * j, 0),
                            )
                    for j in range(4):
                        nc.vector.tensor_mul(
                            VPf[j][:, s * SUBW:(s + 1) * SUBW],
                            vt[j][:],
                            p_ct[:, s * SUBW:(s + 1) * SUBW],
                        )
                return VPf

            def emit_agr_quad(g, VPf):
                """Smat reduce + exp + Z-chain folds for quad g."""
                for j in range(4):
                    f = 4 * g + j
                    for s in range(NSUB):
                        ag = agp.tile([128, SUBW], F32, tag="agr",
                                      name=f"ag{g}{j}{s}")
                        nc.tensor.matmul(
                            ag[:],
                            smat[:],
                            VPf[j][:, s * SUBW:(s + 1) * SUBW],
                            start=True,
                            stop=True,
                        )
                        nc.scalar.activation(
                            e_all[:, f * SLAB + s * SUBW:
                                  f * SLAB + (s + 1) * SUBW],
                            ag[:],
                            AF.Exp,
                        )
                for j in range(4):
                    f = 4 * g + j
                    ef = e_all[:, f * SLAB:(f + 1) * SLAB]
                    if f < 16:
                        if f == 0:
                            nc.gpsimd.tensor_copy(Zlo[:], ef)
                        else:
                            nc.gpsimd.tensor_add(Zlo[:], Zlo[:], ef)
                    else:
                        if f == 16:
                            nc.vector.tensor_copy(Zacc[:], ef)
                        else:
                            nc.vector.tensor_add(Zacc[:], Zacc[:], ef)

            # software-pipelined: quad g's reduce work is emitted after
            # quad g+1's V-matmuls so the in-order Tensor queue never
            # stalls waiting on the DVE VP muls.
            pend = None
            for g in range(8):
                VPf = emit_v_quad(g)
                if pend is not None:
                    emit_agr_quad(g - 1, pend)
                pend = VPf
            emit_agr_quad(7, pend)
            # Zr = 1/Z as exp(-ln(Z)) on ACT (Reciprocal AF is banned and the
            # DVE iterative reciprocal costs ~8us at this size). Merge, Ln,
            # Exp, pp run per 432-col sub so the chain pipelines.
            zln = work.tile([128, SLAB], F32, tag="zln")
            Zr = work.tile([128, SLAB], F16, tag="Zr")
            pp = work.tile([128, SLAB], F16, tag="pp")
            for s in range(NSUB):
                sl = slice(s * SUBW, (s + 1) * SUBW)
                nc.vector.tensor_add(Zacc[:, sl], Zacc[:, sl], Zlo[:, sl])
                nc.scalar.activation(zln[:, sl], Zacc[:, sl], AF.Ln)
                nc.scalar.activation(Zr[:, sl], zln[:, sl], AF.Exp, scale=-1.0)
                nc.vector.tensor_mul(pp[:, sl], p_ct[:, sl], Zr[:, sl])

            # ---------------- pass B: S, centroids ----------------
            cen_sb = work.tile([128, 8 * NPOS], F32, tag="cen_sb")
            gat = work.tile([NPOS, F * DO], F32, tag="gat")
            sq2 = work.tile([NPOS, F * DO], F32, tag="sq2")
            sn2 = work.tile([NPOS, F], F32, tag="sn2")

            def emit_s_quad(g):
                S = [
                    ring.tile([128, SLAB], F16, tag="S", bufs=12,
                              name=f"S{g}_{j}")
                    for j in range(4)
                ]
                for j in range(4):
                    f = 4 * g + j
                    nc.vector.tensor_mul(
                        S[j][:], pp[:], e_all[:, f * SLAB:(f + 1) * SLAB]
                    )
                return S

            def emit_cen_quad(g, S):
                cg = vps.tile([128, NPOS], F32, tag="vps", name=f"cg{g}")
                for t in range(NT):
                    for j in range(4):
                        f = 4 * g + j
                        nc.tensor.matmul(
                            cg[32 * j:32 * j + DO, :],
                            w_r[:, t * F * DO + f * DO:t * F * DO + (f + 1) * DO],
                            S[j][:, t * NPOS:(t + 1) * NPOS],
                            start=(t == 0),
                            stop=(t == NT - 1),
                            tile_position=(0, 32 * j),
                        )
                nc.scalar.copy(cen_sb[:, g * NPOS:(g + 1) * NPOS], cg[:])
                # transpose quad g's centroids while quad g+1's matmuls run
                tp2 = agp.tile([128, 128], F32, tag="agr", name=f"tp2_{g}")
                nc.tensor.transpose(
                    tp2[0:NPOS, :], cen_sb[:, g * NPOS:(g + 1) * NPOS],
                    eye128f[:]
                )
                nc.scalar.copy(
                    gat[:, g * 4 * DO:(g + 1) * 4 * DO]
                    .rearrange("p (j o) -> p j o", o=DO),
                    tp2[0:NPOS, :].rearrange("p (j s) -> p j s", s=32)[:, :, 0:DO],
                )
                # squash2 front half for this quad (overlaps later quads)
                gsl = slice(g * 4 * DO, (g + 1) * 4 * DO)
                nc.scalar.activation(sq2[:, gsl], gat[:, gsl], AF.Square)
                nc.vector.reduce_sum(
                    sn2[:, 4 * g:4 * (g + 1)],
                    sq2[:, gsl].rearrange("p (f o) -> p f o", o=DO),
                    axis=AX.X,
                )

            # two-quad-ahead pipeline: S muls on DVE run while earlier
            # quads' cen matmuls stream on the Tensor engine.
            s_tiles = [emit_s_quad(0), emit_s_quad(1)]
            for g in range(8):
                if g + 2 < 8:
                    s_tiles.append(emit_s_quad(g + 2))
                emit_cen_quad(g, s_tiles[g])

            # ---------------- squash2 tail (sn2 already reduced) ----------
            t1 = work.tile([NPOS, F], F32, tag="q_t1")
            nc.vector.tensor_scalar_add(t1[:], sn2[:], 1.0)
            r1 = work.tile([NPOS, F], F32, tag="q_r1")
            nc.vector.reciprocal(r1[:], t1[:])
            se = work.tile([NPOS, F], F32, tag="q_se")
            nc.vector.tensor_scalar_add(se[:], sn2[:], EPS)
            r2 = work.tile([NPOS, F], F32, tag="q_r2")
            nc.scalar.activation(r2[:], se[:], AF.Sqrt)
            r3 = work.tile([NPOS, F], F32, tag="q_r3")
            nc.vector.reciprocal(r3[:], r2[:])
            sc = work.tile([NPOS, F], F32, tag="q_sc")
            nc.vector.tensor_mul(sc[:], sn2[:], r1[:])
            sc2 = work.tile([NPOS, F], F32, tag="q_sc2")
            nc.vector.tensor_mul(sc2[:], sc[:], r3[:])
            y_sb = work.tile([NPOS, F * DO], F32, tag="y_sb")
            nc.vector.tensor_mul(
                y_sb[:].rearrange("p (f o) -> p f o", o=DO),
                gat[:].rearrange("p (f o) -> p f o", o=DO),
                sc2[:].unsqueeze(2).broadcast_to((NPOS, F, DO)),
            )
            nc.sync.dma_start(y_d[:], y_sb[:])

    nc.compile()
    return nc


def _get_program():
    if "nc" not in _CACHE:
        _CACHE["nc"] = _build()
    return _CACHE["nc"]


def _host_inputs(x, Wm):
    w_r, w_v = _host_weights(Wm)
    smat = np.kron(np.eye(CB), np.ones((DI, DI))).astype(np.float16)
    eye72 = np.eye(NPOS, dtype=np.float16)
    eye128f = np.eye(128, dtype=np.float32)
    in_maps = []
    for k in range(8):
        in_maps.append({
            "p_ct": _host_patches(x, k),
            "w_r": w_r,
            "w_v": w_v,
            "smat": smat,
            "eye72": eye72,
            "eye128f": eye128f,
        })
    return in_maps


def kernel(x, W):
    from concourse.bass_utils import run_bass_kernel_spmd

    x = np.asarray(x, np.float32)
    Wm = np.asarray(W, np.float32)[0, 0, 0]
    nc = _get_program()
    in_maps = _host_inputs(x, Wm)
    res = run_bass_kernel_spmd(nc, in_maps, list(range(8)))
    Ho, Wo = H - KH + 1, WD - KW + 1
    y = np.empty((B, Ho, Wo, F, DO), np.float32)
    for k in range(8):
        b, hh = divmod(k, 2)
        y[b, 6 * hh:6 * hh + 6] = res.results[k]["y"].reshape(6, Wo, F, DO)
    return y
